# revision 28
# baseline (speedup 1.0000x reference)
"""Local (sliding-window) attention kernel for Trainium2, 8 NeuronCores.

Problem: x [B=2, L=2048, E=512] fp32; q/k/v = x @ W{q,k,v}.T + b; scores over a
+-64 window, softmax, out = probs @ v_win.

Sharding: 8 cores = (batch 2) x (4 sequence chunks of 512 queries). Each core
gets a transposed, halo'd slice xT [E, 640] (64 halo keys each side,
zero-padded at sequence ends) and computes its own q/k/v projections
(weights replicated), then 4 blocks of 128 queries x 256-key-span windowed
attention. Matmul inputs are fp16 (PSUM accumulates fp32).

PE stream: short warm-up (HAM clock ramp) -> q proj -> k proj -> v proj ->
scores (4 blocks) -> transposes -> AV per block. q/k are ec-outer so each
gates only on chunk-0 DMAs. PSUM banks: pool "mm" (warm, q x4, k-half1 x4,
v x5, o x4) + pool "ss" (k-half0 x4, scores x4, probs-transpose x4) = 8, laid
out so no matmul ever waits on a PSUM drain by a slow engine: k half0 lands in
the (idle) score banks instead of waiting for the serialized q-bias ACTs.

Softmax: exp on Scalar with fused row-sum (accum_out), reciprocal on DVE; the
in-band window mask is an additive -1e4 folded into the scores matmul via an
identity matmul (one [128,256] band mask shared by all blocks). Sequence-
boundary clipping is NOT masked: padded x rows are exact zeros, so clipped
keys score exactly exp(0)=1 and contribute v_pad=0 to AV; only the softmax
denominator needs the host-precomputed per-query count of in-band
out-of-sequence keys subtracted (requires bk == 0 and bv == 0, asserted --
true for this problem). Output 1/r scaling is split Scalar/DVE; out DMA'd in
fp16 (host upcasts).

DMA: inputs are host-repacked to [128, big-row] layouts (>=3.8KB rows) and
issued on FOUR queues (Sync: xt; GpSimd: wq, wv; Vector: wk; Scalar: band
mask, misc) in first-need order -- DMA issue occupies an engine ~0.7-1.1us
per descriptor, so spreading issues keeps the 16 DMA engines fed. Output
DMAs alternate Sync/GpSimd.
"""

import numpy as np

B, L, E = 2, 2048, 512
WHALF = 64
NCORES = 8
CHUNK = 512            # queries per core
SPAN = CHUNK + 2 * WHALF   # 640 key/value positions per core
BLK = 128              # query block
NBLK = CHUNK // BLK    # 4
KSPAN = 2 * BLK        # 256-key span per query block
EC = E // 128          # 4 e-chunks
N_WARM = 13            # warm-up matmuls (HAM ramp + cover DMA latency; long
                       # enough that q+k never stall on DMAs mid-stream,
                       # which would trigger a HAM down-clock)
MASK_NEG = -10000.0    # additive mask value (pre exp-scale)

_CACHE = {}


def _build_bass():
    import concourse.bass as bass
    import concourse.mybir as mybir
    from concourse.tile import TileContext

    f32 = mybir.dt.float32
    f16 = mybir.dt.float16
    AF = mybir.ActivationFunctionType

    nc = bass.Bass()
    # host-packed inputs: [partition, chunk-major big rows]
    xtp = nc.dram_tensor("xtp", [128, EC * SPAN], f16, kind="ExternalInput")
    wqp = nc.dram_tensor("wqp", [128, EC * E], f16, kind="ExternalInput")
    wkp = nc.dram_tensor("wkp", [128, EC * E], f16, kind="ExternalInput")
    wvp = nc.dram_tensor("wvp", [128, EC * E], f16, kind="ExternalInput")
    # misc per-partition scalars: [p, 2*c+{0,1}] = bq/bk pairs, [p, 8+i] = ninv
    misc = nc.dram_tensor("misc", [128, 2 * EC + NBLK], f32, kind="ExternalInput")
    # band mask (additive, 0 / -1e4; shared by all blocks) + 128x128 identity
    mi = nc.dram_tensor("mi", [128, KSPAN + BLK], f16, kind="ExternalInput")
    # block-major output layout [partition, block, e] so out DMAs move 2KB+
    # contiguous rows per partition (host transposes back)
    out = nc.dram_tensor("out", [128, NBLK, E], f16, kind="ExternalOutput")
    # Dummy output that keeps the PE warm-up matmul stream live (not read by
    # the host). HAM throttles TensorE until a few us of sustained activity;
    # the warm-up bridges the gap until the first input DMAs land.
    warm_out = nc.dram_tensor("warm_out", [128, 16], f32, kind="ExternalOutput")

    inv_sqrt_e = float(1.0 / np.sqrt(E))

    with TileContext(nc) as tc:
        with tc.tile_pool(name="sb", bufs=1) as sb, \
             tc.tile_pool(name="ps", bufs=4, space="PSUM") as ps, \
             tc.tile_pool(name="pss", bufs=4, space="PSUM") as pss:
            # ---------- input DMAs, four queues, first-need order ----------
            xt = sb.tile([128, EC, SPAN], f16)
            wq = sb.tile([128, EC, E], f16)
            wk = sb.tile([128, EC, E], f16)
            wv = sb.tile([128, EC, E], f16)
            misc_t = sb.tile([128, 2 * EC + NBLK], f32)
            mi_t = sb.tile([128, KSPAN + BLK], f16)
            wrm = sb.tile([128, E], f16)

            # DMA rate scales with row size (~193GB/s at 2KB rows vs ~63 at
            # 1KB), so everything moves in chunk-PAIR transfers. Two HWDGE
            # queues split the early-critical stream: Sync xt+wk, Scalar
            # wq+misc+mask; SWDGE (GpSimd) carries only wv (needed last).
            nc.vector.memset(wrm[:], 0.0)
            # Sync (FIFO): xt pairs (gate q's ec groups), wk c0+c1, small
            for h in range(2):
                nc.sync.dma_start(
                    out=xt[:, 2 * h:2 * h + 2, :],
                    in_=xtp[:, 2 * h * SPAN:(2 * h + 2) * SPAN].rearrange(
                        "p (c j) -> p c j", c=2))
            nc.sync.dma_start(
                out=wk[:, 0:2, :],
                in_=wkp[:, 0:2 * E].rearrange("p (c e) -> p c e", c=2))
            nc.sync.dma_start(out=misc_t[:], in_=misc[:])
            nc.sync.dma_start(out=mi_t[:], in_=mi[:])
            # Scalar (FIFO): wq pairs
            for h in range(2):
                nc.scalar.dma_start(
                    out=wq[:, 2 * h:2 * h + 2, :],
                    in_=wqp[:, 2 * h * E:(2 * h + 2) * E].rearrange(
                        "p (c e) -> p c e", c=2))
            # GpSimd (SWDGE round-robins its queue, no FIFO order): wk c2+c3
            # and wv, needed only from ~17us on.
            nc.gpsimd.dma_start(
                out=wk[:, 2:4, :],
                in_=wkp[:, 2 * E:4 * E].rearrange("p (c e) -> p c e", c=2))
            nc.gpsimd.dma_start(
                out=wv[:], in_=wvp.rearrange("p (c e) -> p c e", c=EC))

            band = mi_t[:, 0:KSPAN]
            idt = mi_t[:, KSPAN:KSPAN + BLK]

            def bias_q(fc):
                return misc_t[:, 2 * fc:2 * fc + 1]

            def bias_k(fc):
                return misc_t[:, 2 * fc + 1:2 * fc + 2]

            def ninv(i):
                return misc_t[:, 2 * EC + i:2 * EC + i + 1]

            # ---------- PE warm-up stream (no input deps) ----------
            # two alternating PSUM banks so the stream is gapless (a denser
            # activity signal for the HAM clock-ramp trigger)
            w_ps = [ps.tile([128, E], f32, tag="mm", name=f"warm{t}")
                    for t in range(2)]
            for n in range(N_WARM):
                nc.tensor.matmul(w_ps[n % 2][:], wrm[:, 0:128], wrm[:],
                                 start=True, stop=True)
            w_sb = sb.tile([128, 16], f32)
            nc.vector.tensor_copy(w_sb[:], w_ps[(N_WARM - 1) % 2][:, 0:16])
            nc.gpsimd.dma_start(out=warm_out[:], in_=w_sb[:])

            # ---------- q projection: qT [e_out, l] fp16 ----------
            # ec-outer over 4 concurrent PSUM groups so the first matmuls only
            # need chunk-0 DMAs. Bias+copy to SBUF on Scalar (ACT).
            qt = sb.tile([128, EC, CHUNK], f16)
            q_ps = [ps.tile([128, CHUNK], f32, tag="mm", name=f"qps{fc}")
                    for fc in range(EC)]
            for ec in range(EC):
                for fc in range(EC):
                    nc.tensor.matmul(
                        q_ps[fc][:],
                        wq[:, ec, fc * 128:(fc + 1) * 128],
                        xt[:, ec, WHALF:WHALF + CHUNK],
                        start=(ec == 0), stop=(ec == EC - 1))
            for fc in range(EC):
                nc.scalar.activation(qt[:, fc, :], q_ps[fc][:], AF.Identity,
                                     bias=bias_q(fc))

            # ---------- k projection: kT [e_out, j] over full 640 span ----------
            # split 640 = 2 x 320 (psum bank limit), ec-outer like q. Half 0
            # accumulates in the still-idle "ss" banks so it never waits on
            # the serialized q-bias ACTs draining the "mm" banks; half 1 (a
            # k-pipeline-depth later) reuses the q banks. Bias+copy on DVE.
            kt = sb.tile([128, EC, SPAN], f16)
            for half in range(2):
                j0 = half * 320
                pool = pss if half == 0 else ps
                tag = "ss" if half == 0 else "mm"
                k_ps = [pool.tile([128, 320], f32, tag=tag, name=f"kps{half}_{fc}")
                        for fc in range(EC)]
                for ec in range(EC):
                    for fc in range(EC):
                        nc.tensor.matmul(
                            k_ps[fc][:],
                            wk[:, ec, fc * 128:(fc + 1) * 128],
                            xt[:, ec, j0:j0 + 320],
                            start=(ec == 0), stop=(ec == EC - 1))
                for fc in range(EC):
                    nc.vector.tensor_scalar_add(
                        kt[:, fc, j0:j0 + 320], k_ps[fc][:], bias_k(fc))

            # ---------- v projection: natural [j, f] layout ----------
            # PSUM->SBUF copies on Scalar (ACT) to keep DVE free for the
            # attention phase.
            v_sb = sb.tile([128, SPAN // 128, E], f16)
            for wave in ([0, 1, 2, 3], [4]):
                v_ps = {jc: ps.tile([128, E], f32, tag="mm", name=f"vps{jc}")
                        for jc in wave}
                for ec in range(EC):
                    for jc in wave:
                        nc.tensor.matmul(
                            v_ps[jc][:],
                            xt[:, ec, jc * 128:(jc + 1) * 128],
                            wv[:, ec, :],
                            start=(ec == 0), stop=(ec == EC - 1))
                for jc in wave:
                    nc.scalar.activation(v_sb[:, jc, :], v_ps[jc][:], AF.Copy)

            # ---------- scores for all 4 blocks ----------
            # s = band + sum_ec qT.T @ kT ; the band mask lands via an identity
            # matmul as the first accumulation step (idt.T @ band == band).
            s_tiles = []
            for i in range(NBLK):
                s_ps = pss.tile([128, KSPAN], f32, tag="ss", name=f"sps{i}")
                nc.tensor.matmul(s_ps[:], idt, band, start=True, stop=False)
                for ec in range(EC):
                    nc.tensor.matmul(
                        s_ps[:],
                        qt[:, ec, i * BLK:(i + 1) * BLK],
                        kt[:, ec, i * BLK:i * BLK + KSPAN],
                        start=False, stop=(ec == EC - 1))
                s_tiles.append(s_ps)

            # exp + fused row-sum on Scalar; no max-subtraction (scores are
            # O(1): x~N(0,1), W~0.02 scale).
            # rowsum correction: clipped-but-in-band keys contribute exactly
            # 1.0 each (padded x is zero, bk==0) -> subtract the host-counted
            # ninv before the reciprocal. Emitted here (not in the AV loop) so
            # the DVE queue is drained before the output scales need it.
            e_sbs, rinvs = [], []
            for i in range(NBLK):
                e_sb = sb.tile([128, KSPAN], f16, tag="esb", name=f"esb{i}", bufs=4)
                r = sb.tile([128, 1], f32, tag="r", name=f"r{i}", bufs=4)
                nc.scalar.activation(e_sb[:], s_tiles[i][:], AF.Exp,
                                     scale=inv_sqrt_e, accum_out=r[:])
                rv = sb.tile([128, 1], f32, tag="rv", name=f"rv{i}", bufs=4)
                nc.vector.tensor_scalar_sub(rv[:], r[:], ninv(i))
                rinv = sb.tile([128, 1], f32, tag="rinv", name=f"rinv{i}", bufs=4)
                nc.vector.reciprocal(rinv[:], rv[:])
                e_sbs.append(e_sb)
                rinvs.append(rinv)

            # ---------- transpose probs (PE), then AV per block ----------
            # T runs one block ahead of AV so each pt copy (DVE) lands before
            # the AV that needs it, and the DVE queue stays drained ahead of
            # the output half-scales.
            o_pair = [sb.tile([128, 2, E], f16, tag="osb", name=f"osb{t}",
                              bufs=2) for t in range(2)]
            pt_sbs = []

            def emit_t(i):
                pt_ps = pss.tile([128, 2, BLK], f16, tag="ss", name=f"ptps{i}")
                nc.tensor.transpose(pt_ps[:, 0, :], e_sbs[i][:, 0:BLK], idt)
                nc.tensor.transpose(pt_ps[:, 1, :], e_sbs[i][:, BLK:KSPAN], idt)
                pt_sb = sb.tile([128, 2, BLK], f16, tag="ptsb", name=f"ptsb{i}", bufs=4)
                nc.vector.tensor_copy(pt_sb[:], pt_ps[:])
                pt_sbs.append(pt_sb)

            emit_t(0)
            emit_t(1)
            for i in range(NBLK):
                if i + 2 < NBLK + 2 and i + 2 <= NBLK - 1:
                    emit_t(i + 2)
                o_ps = ps.tile([128, E], f32, tag="mm", name=f"ops{i}")
                nc.tensor.matmul(o_ps[:], pt_sbs[i][:, 0, :], v_sb[:, i, :],
                                 start=True, stop=False)
                nc.tensor.matmul(o_ps[:], pt_sbs[i][:, 1, :], v_sb[:, i + 1, :],
                                 start=False, stop=True)
                # output 1/r scale split across Scalar and DVE halves
                o_sb = o_pair[i // 2][:, i % 2, :]
                nc.scalar.activation(o_sb[0:128, 0:E // 2], o_ps[:, 0:E // 2],
                                     AF.Copy, scale=rinvs[i][:])
                nc.vector.tensor_scalar_mul(o_sb[0:128, E // 2:E],
                                            o_ps[:, E // 2:E], rinvs[i][:])
                if i % 2 == 1:
                    # last pair rides Sync: it's idle by then and fastest
                    eng = nc.scalar if i == 1 else nc.sync
                    eng.dma_start(out=out[:, i - 1:i + 1, :],
                                  in_=o_pair[i // 2][:])

    _split_multi_waits(nc)
    return nc


def _split_multi_waits(nc):
    """This walrus build accepts only ONE sync wait per engine instruction;
    Tile emits 2+ on phase-crossing instructions. Peel extra waits onto
    same-engine NoOps placed immediately before (engine streams are in-order,
    so the waits still guard the instruction)."""
    import concourse.mybir as mybir

    for fn in nc.m.functions:
        for blk in fn.blocks:
            new_insts = []
            for inst in blk.instructions:
                si = inst.sync_info
                waits = list(si.on_wait) if si is not None and si.on_wait else []
                if len(waits) > 1:
                    for w in waits[:-1]:
                        new_insts.append(mybir.InstNoOp(
                            name=nc.get_next_instruction_name(),
                            engine=inst.engine,
                            ins=[], outs=[],
                            sync_info=mybir.SyncInfo(on_wait=[w], on_update=[]),
                        ))
                    inst.sync_info = mybir.SyncInfo(
                        on_wait=[waits[-1]], on_update=list(si.on_update or []))
                new_insts.append(inst)
            blk.instructions = new_insts


def _host_inputs(x, Wq, bq, Wk, bk, Wv, bv):
    # weights packed chunk-major: [p, c*E + e] = W.T[c*128+p, e]
    def packw(W):
        wt = np.ascontiguousarray(W.T).astype(np.float16)  # [E_in, E_out]
        return np.ascontiguousarray(
            wt.reshape(EC, 128, E).transpose(1, 0, 2).reshape(128, EC * E))
    wqp, wkp, wvp = packw(Wq), packw(Wk), packw(Wv)
    idn = np.eye(BLK, dtype=np.float16)
    p = np.arange(BLK)[:, None]
    jj = np.arange(KSPAN)[None, :]
    band = (jj >= p) & (jj <= p + 2 * WHALF)
    band_add = np.where(band, np.float16(0.0), np.float16(MASK_NEG))
    mi = np.ascontiguousarray(
        np.concatenate([band_add, idn], axis=1))  # [128, 384]
    in_maps = []
    for c in range(NCORES):
        b, ci = divmod(c, NBLK)
        s = ci * CHUNK
        lo, hi = s - WHALF, s + CHUNK + WHALF
        a0, a1 = max(lo, 0), min(hi, L)
        xh = np.zeros((SPAN, E), np.float32)
        xh[a0 - lo:a1 - lo] = x[b, a0:a1]
        xT = np.ascontiguousarray(xh.T).astype(np.float16)  # [E, SPAN]
        xtp = np.ascontiguousarray(
            xT.reshape(EC, 128, SPAN).transpose(1, 0, 2).reshape(128, EC * SPAN))
        misc = np.zeros((128, 2 * EC + NBLK), np.float32)
        misc[:, 0:2 * EC:2] = bq.reshape(EC, 128).T
        misc[:, 1:2 * EC:2] = bk.reshape(EC, 128).T
        for i in range(NBLK):
            g = s - WHALF + i * BLK + jj  # global key index [1, KSPAN]
            n_invalid = (band & ((g < 0) | (g >= L))).sum(axis=1)
            misc[:, 2 * EC + i] = n_invalid.astype(np.float32)
        in_maps.append({
            "xtp": xtp, "wqp": wqp, "wkp": wkp, "wvp": wvp,
            "misc": misc, "mi": mi,
        })
    return in_maps


def kernel(x, Wq, bq, Wk, bk, Wv, bv, window_size, _trace=False):
    from concourse import bass_utils

    x = np.asarray(x, dtype=np.float32)
    Wq = np.asarray(Wq, dtype=np.float32)
    Wk = np.asarray(Wk, dtype=np.float32)
    Wv = np.asarray(Wv, dtype=np.float32)
    bq = np.asarray(bq, dtype=np.float32)
    bk = np.asarray(bk, dtype=np.float32)
    bv = np.asarray(bv, dtype=np.float32)
    assert int(window_size) == WHALF, f"kernel hardcodes window_size={WHALF}"
    assert x.shape == (B, L, E)
    # boundary-clip handling relies on padded keys scoring exp(0)=1 with zero
    # value vectors; that needs zero k/v biases (true for this problem).
    assert not np.any(bk) and not np.any(bv), "kernel requires bk == bv == 0"

    if "nc" not in _CACHE:
        _CACHE["nc"] = _build_bass()
    nc = _CACHE["nc"]

    in_maps = _host_inputs(x, Wq, bq, Wk, bk, Wv, bv)
    res = bass_utils.run_bass_kernel_spmd(
        nc, in_maps, core_ids=list(range(NCORES)), trace=_trace)
    _CACHE["last_results"] = res

    out = np.empty((B, L, E), np.float32)
    for c in range(NCORES):
        b, ci = divmod(c, NBLK)
        blk = res.results[c]["out"]  # [128, NBLK, E] block-major
        out[b, ci * CHUNK:(ci + 1) * CHUNK] = (
            blk.transpose(1, 0, 2).reshape(CHUNK, E).astype(np.float32))
    return out


# revision 30
# speedup vs baseline: 1.1435x; 1.1435x over previous
"""Local (sliding-window) attention kernel for Trainium2, 8 NeuronCores.

Problem: x [B=2, L=2048, E=512] fp32; q/k/v = x @ W{q,k,v}.T + b; scores over a
+-64 window, softmax, out = probs @ v_win.

Sharding: 8 cores = (batch 2) x (4 sequence chunks of 512 queries). Each core
gets a transposed, halo'd slice xT [E, 640] (64 halo keys each side,
zero-padded at sequence ends) and computes its own q/k/v projections
(weights replicated), then 4 blocks of 128 queries x 256-key-span windowed
attention. Matmul inputs are fp16 (PSUM accumulates fp32).

PE stream: short warm-up (HAM clock ramp) -> q proj -> k proj -> v proj ->
scores (4 blocks) -> transposes -> AV per block. q/k are ec-outer so each
gates only on chunk-0 DMAs. PSUM banks: pool "mm" (warm, q x4, k-half1 x4,
v x5, o x4) + pool "ss" (k-half0 x4, scores x4, probs-transpose x4) = 8, laid
out so no matmul ever waits on a PSUM drain by a slow engine: k half0 lands in
the (idle) score banks instead of waiting for the serialized q-bias ACTs.

Softmax: exp on Scalar with fused row-sum (accum_out), reciprocal on DVE; the
in-band window mask is an additive -1e4 folded into the scores matmul via an
identity matmul (one [128,256] band mask shared by all blocks). Sequence-
boundary clipping is NOT masked: padded x rows are exact zeros, so clipped
keys score exactly exp(0)=1 and contribute v_pad=0 to AV; only the softmax
denominator needs the host-precomputed per-query count of in-band
out-of-sequence keys subtracted (requires bk == 0 and bv == 0, asserted --
true for this problem). Output 1/r scaling is split Scalar/DVE; out DMA'd in
fp16 (host upcasts).

DMA: inputs are host-repacked to [128, big-row] layouts (>=3.8KB rows) and
issued on FOUR queues (Sync: xt; GpSimd: wq, wv; Vector: wk; Scalar: band
mask, misc) in first-need order -- DMA issue occupies an engine ~0.7-1.1us
per descriptor, so spreading issues keeps the 16 DMA engines fed. Output
DMAs alternate Sync/GpSimd.
"""

import numpy as np

B, L, E = 2, 2048, 512
WHALF = 64
NCORES = 8
CHUNK = 512            # queries per core
SPAN = CHUNK + 2 * WHALF   # 640 key/value positions per core
BLK = 128              # query block
NBLK = CHUNK // BLK    # 4
KSPAN = 2 * BLK        # 256-key span per query block
EC = E // 128          # 4 e-chunks
N_WARM = 9             # warm-up matmuls (HAM ramp + cover DMA latency; ends
                       # right when the first q operands land)
MASK_NEG = -10000.0    # additive mask value (pre exp-scale)

_CACHE = {}


def _build_bass():
    import concourse.bass as bass
    import concourse.mybir as mybir
    from concourse.tile import TileContext

    f32 = mybir.dt.float32
    f16 = mybir.dt.float16
    AF = mybir.ActivationFunctionType

    nc = bass.Bass()
    # host-packed inputs: [partition, chunk-major big rows]
    xtp = nc.dram_tensor("xtp", [128, EC * SPAN], f16, kind="ExternalInput")
    wqp = nc.dram_tensor("wqp", [128, EC * E], f16, kind="ExternalInput")
    wkp = nc.dram_tensor("wkp", [128, EC * E], f16, kind="ExternalInput")
    wvp = nc.dram_tensor("wvp", [128, EC * E], f16, kind="ExternalInput")
    # misc per-partition scalars: [p, 2*c+{0,1}] = bq/bk pairs, [p, 8+i] = ninv
    misc = nc.dram_tensor("misc", [128, 2 * EC + NBLK], f32, kind="ExternalInput")
    # band mask (additive, 0 / -1e4; shared by all blocks) + 128x128 identity
    mi = nc.dram_tensor("mi", [128, KSPAN + BLK], f16, kind="ExternalInput")
    # block-major output layout [partition, block, e] so out DMAs move 2KB+
    # contiguous rows per partition (host transposes back)
    out = nc.dram_tensor("out", [128, NBLK, E], f16, kind="ExternalOutput")
    # Dummy output that keeps the PE warm-up matmul stream live (not read by
    # the host). HAM throttles TensorE until a few us of sustained activity;
    # the warm-up bridges the gap until the first input DMAs land.
    warm_out = nc.dram_tensor("warm_out", [128, 16], f32, kind="ExternalOutput")

    inv_sqrt_e = float(1.0 / np.sqrt(E))

    with TileContext(nc) as tc:
        with tc.tile_pool(name="sb", bufs=1) as sb, \
             tc.tile_pool(name="ps", bufs=4, space="PSUM") as ps, \
             tc.tile_pool(name="pss", bufs=4, space="PSUM") as pss:
            # ---------- input DMAs, four queues, first-need order ----------
            xt = sb.tile([128, EC, SPAN], f16)
            wq = sb.tile([128, EC, E], f16)
            wk = sb.tile([128, EC, E], f16)
            wv = sb.tile([128, EC, E], f16)
            misc_t = sb.tile([128, 2 * EC + NBLK], f32)
            mi_t = sb.tile([128, KSPAN + BLK], f16)
            wrm = sb.tile([128, E], f16)

            # DMA rate scales with row size (~193GB/s at 2KB rows vs ~63 at
            # 1KB), so everything moves in chunk-PAIR transfers. Two HWDGE
            # queues split the early-critical stream: Sync xt+wk, Scalar
            # wq+misc+mask; SWDGE (GpSimd) carries only wv (needed last).
            nc.vector.memset(wrm[:], 0.0)
            # x and wq are the early-critical stream; split them across BOTH
            # fast FIFO queues so neither queue carries more than ~600KB
            # before q's last-needed chunk.
            # Sync (FIFO): xt c0+c1, wq c2+c3, wk c0+c1
            nc.sync.dma_start(
                out=xt[:, 0:2, :],
                in_=xtp[:, 0:2 * SPAN].rearrange("p (c j) -> p c j", c=2))
            nc.sync.dma_start(
                out=wq[:, 2:4, :],
                in_=wqp[:, 2 * E:4 * E].rearrange("p (c e) -> p c e", c=2))
            nc.sync.dma_start(
                out=wk[:, 0:2, :],
                in_=wkp[:, 0:2 * E].rearrange("p (c e) -> p c e", c=2))
            # Scalar (FIFO): wq c0+c1, xt c2+c3
            nc.scalar.dma_start(
                out=wq[:, 0:2, :],
                in_=wqp[:, 0:2 * E].rearrange("p (c e) -> p c e", c=2))
            nc.scalar.dma_start(
                out=xt[:, 2:4, :],
                in_=xtp[:, 2 * SPAN:4 * SPAN].rearrange("p (c j) -> p c j", c=2))
            # GpSimd (SWDGE round-robins its queue, no FIFO order): wk c2+c3,
            # wv, misc, mask -- all needed from ~15us on, all done by then.
            nc.gpsimd.dma_start(
                out=wk[:, 2:4, :],
                in_=wkp[:, 2 * E:4 * E].rearrange("p (c e) -> p c e", c=2))
            nc.gpsimd.dma_start(
                out=wv[:], in_=wvp.rearrange("p (c e) -> p c e", c=EC))
            nc.gpsimd.dma_start(out=misc_t[:], in_=misc[:])
            nc.gpsimd.dma_start(out=mi_t[:], in_=mi[:])

            band = mi_t[:, 0:KSPAN]
            idt = mi_t[:, KSPAN:KSPAN + BLK]

            def bias_q(fc):
                return misc_t[:, 2 * fc:2 * fc + 1]

            def bias_k(fc):
                return misc_t[:, 2 * fc + 1:2 * fc + 2]

            def ninv(i):
                return misc_t[:, 2 * EC + i:2 * EC + i + 1]

            # ---------- PE warm-up stream (no input deps) ----------
            # two alternating PSUM banks so the stream is gapless (a denser
            # activity signal for the HAM clock-ramp trigger)
            w_ps = [ps.tile([128, E], f32, tag="mm", name=f"warm{t}")
                    for t in range(2)]
            for n in range(N_WARM):
                nc.tensor.matmul(w_ps[n % 2][:], wrm[:, 0:128], wrm[:],
                                 start=True, stop=True)
            w_sb = sb.tile([128, 16], f32)
            nc.vector.tensor_copy(w_sb[:], w_ps[(N_WARM - 1) % 2][:, 0:16])
            nc.gpsimd.dma_start(out=warm_out[:], in_=w_sb[:])

            # ---------- q projection: qT [e_out, l] fp16 ----------
            # ec-outer over 4 concurrent PSUM groups so the first matmuls only
            # need chunk-0 DMAs. Bias+copy to SBUF on Scalar (ACT).
            qt = sb.tile([128, EC, CHUNK], f16)
            q_ps = [ps.tile([128, CHUNK], f32, tag="mm", name=f"qps{fc}")
                    for fc in range(EC)]
            for ec in range(EC):
                for fc in range(EC):
                    nc.tensor.matmul(
                        q_ps[fc][:],
                        wq[:, ec, fc * 128:(fc + 1) * 128],
                        xt[:, ec, WHALF:WHALF + CHUNK],
                        start=(ec == 0), stop=(ec == EC - 1))
            for fc in range(EC):
                nc.scalar.activation(qt[:, fc, :], q_ps[fc][:], AF.Identity,
                                     bias=bias_q(fc))

            # ---------- k projection: kT [e_out, j] over full 640 span ----------
            # split 640 = 2 x 320 (psum bank limit), ec-outer like q. Half 0
            # accumulates in the still-idle "ss" banks so it never waits on
            # the serialized q-bias ACTs draining the "mm" banks; half 1 (a
            # k-pipeline-depth later) reuses the q banks. Bias+copy on DVE.
            kt = sb.tile([128, EC, SPAN], f16)
            for half in range(2):
                j0 = half * 320
                pool = pss if half == 0 else ps
                tag = "ss" if half == 0 else "mm"
                k_ps = [pool.tile([128, 320], f32, tag=tag, name=f"kps{half}_{fc}")
                        for fc in range(EC)]
                for ec in range(EC):
                    for fc in range(EC):
                        nc.tensor.matmul(
                            k_ps[fc][:],
                            wk[:, ec, fc * 128:(fc + 1) * 128],
                            xt[:, ec, j0:j0 + 320],
                            start=(ec == 0), stop=(ec == EC - 1))
                for fc in range(EC):
                    nc.vector.tensor_scalar_add(
                        kt[:, fc, j0:j0 + 320], k_ps[fc][:], bias_k(fc))

            # ---------- v projection: natural [j, f] layout ----------
            # PSUM->SBUF copies on Scalar (ACT) to keep DVE free for the
            # attention phase.
            v_sb = sb.tile([128, SPAN // 128, E], f16)
            for wave in ([0, 1, 2, 3], [4]):
                v_ps = {jc: ps.tile([128, E], f32, tag="mm", name=f"vps{jc}")
                        for jc in wave}
                for ec in range(EC):
                    for jc in wave:
                        nc.tensor.matmul(
                            v_ps[jc][:],
                            xt[:, ec, jc * 128:(jc + 1) * 128],
                            wv[:, ec, :],
                            start=(ec == 0), stop=(ec == EC - 1))
                for jc in wave:
                    nc.scalar.activation(v_sb[:, jc, :], v_ps[jc][:], AF.Copy)

            # ---------- scores for all 4 blocks ----------
            # s = band + sum_ec qT.T @ kT ; the band mask lands via an identity
            # matmul as the first accumulation step (idt.T @ band == band).
            s_tiles = []
            for i in range(NBLK):
                s_ps = pss.tile([128, KSPAN], f32, tag="ss", name=f"sps{i}")
                nc.tensor.matmul(s_ps[:], idt, band, start=True, stop=False)
                for ec in range(EC):
                    nc.tensor.matmul(
                        s_ps[:],
                        qt[:, ec, i * BLK:(i + 1) * BLK],
                        kt[:, ec, i * BLK:i * BLK + KSPAN],
                        start=False, stop=(ec == EC - 1))
                s_tiles.append(s_ps)

            # exp + fused row-sum on Scalar; no max-subtraction (scores are
            # O(1): x~N(0,1), W~0.02 scale).
            # rowsum correction: clipped-but-in-band keys contribute exactly
            # 1.0 each (padded x is zero, bk==0) -> subtract the host-counted
            # ninv before the reciprocal. Emitted here (not in the AV loop) so
            # the DVE queue is drained before the output scales need it.
            e_sbs, rinvs = [], []
            for i in range(NBLK):
                e_sb = sb.tile([128, KSPAN], f16, tag="esb", name=f"esb{i}", bufs=4)
                r = sb.tile([128, 1], f32, tag="r", name=f"r{i}", bufs=4)
                nc.scalar.activation(e_sb[:], s_tiles[i][:], AF.Exp,
                                     scale=inv_sqrt_e, accum_out=r[:])
                rv = sb.tile([128, 1], f32, tag="rv", name=f"rv{i}", bufs=4)
                nc.vector.tensor_scalar_sub(rv[:], r[:], ninv(i))
                rinv = sb.tile([128, 1], f32, tag="rinv", name=f"rinv{i}", bufs=4)
                nc.vector.reciprocal(rinv[:], rv[:])
                e_sbs.append(e_sb)
                rinvs.append(rinv)

            # ---------- transpose probs (PE), then AV per block ----------
            # T runs one block ahead of AV so each pt copy (DVE) lands before
            # the AV that needs it, and the DVE queue stays drained ahead of
            # the output half-scales.
            o_pair = [sb.tile([128, 2, E], f16, tag="osb", name=f"osb{t}",
                              bufs=2) for t in range(2)]
            pt_sbs = []

            def emit_t(i):
                pt_ps = pss.tile([128, 2, BLK], f16, tag="ss", name=f"ptps{i}")
                nc.tensor.transpose(pt_ps[:, 0, :], e_sbs[i][:, 0:BLK], idt)
                nc.tensor.transpose(pt_ps[:, 1, :], e_sbs[i][:, BLK:KSPAN], idt)
                pt_sb = sb.tile([128, 2, BLK], f16, tag="ptsb", name=f"ptsb{i}", bufs=4)
                nc.vector.tensor_copy(pt_sb[:], pt_ps[:])
                pt_sbs.append(pt_sb)

            emit_t(0)
            emit_t(1)
            for i in range(NBLK):
                if i + 2 < NBLK + 2 and i + 2 <= NBLK - 1:
                    emit_t(i + 2)
                o_ps = ps.tile([128, E], f32, tag="mm", name=f"ops{i}")
                nc.tensor.matmul(o_ps[:], pt_sbs[i][:, 0, :], v_sb[:, i, :],
                                 start=True, stop=False)
                nc.tensor.matmul(o_ps[:], pt_sbs[i][:, 1, :], v_sb[:, i + 1, :],
                                 start=False, stop=True)
                # output 1/r scale split across Scalar and DVE halves
                o_sb = o_pair[i // 2][:, i % 2, :]
                nc.scalar.activation(o_sb[0:128, 0:E // 2], o_ps[:, 0:E // 2],
                                     AF.Copy, scale=rinvs[i][:])
                nc.vector.tensor_scalar_mul(o_sb[0:128, E // 2:E],
                                            o_ps[:, E // 2:E], rinvs[i][:])
                if i % 2 == 1:
                    # last pair rides Sync: it's idle by then and fastest
                    eng = nc.scalar if i == 1 else nc.sync
                    eng.dma_start(out=out[:, i - 1:i + 1, :],
                                  in_=o_pair[i // 2][:])

    _split_multi_waits(nc)
    return nc


def _split_multi_waits(nc):
    """This walrus build accepts only ONE sync wait per engine instruction;
    Tile emits 2+ on phase-crossing instructions. Peel extra waits onto
    same-engine NoOps placed immediately before (engine streams are in-order,
    so the waits still guard the instruction)."""
    import concourse.mybir as mybir

    for fn in nc.m.functions:
        for blk in fn.blocks:
            new_insts = []
            for inst in blk.instructions:
                si = inst.sync_info
                waits = list(si.on_wait) if si is not None and si.on_wait else []
                if len(waits) > 1:
                    for w in waits[:-1]:
                        new_insts.append(mybir.InstNoOp(
                            name=nc.get_next_instruction_name(),
                            engine=inst.engine,
                            ins=[], outs=[],
                            sync_info=mybir.SyncInfo(on_wait=[w], on_update=[]),
                        ))
                    inst.sync_info = mybir.SyncInfo(
                        on_wait=[waits[-1]], on_update=list(si.on_update or []))
                new_insts.append(inst)
            blk.instructions = new_insts


def _host_inputs(x, Wq, bq, Wk, bk, Wv, bv):
    # weights packed chunk-major: [p, c*E + e] = W.T[c*128+p, e]
    def packw(W):
        wt = np.ascontiguousarray(W.T).astype(np.float16)  # [E_in, E_out]
        return np.ascontiguousarray(
            wt.reshape(EC, 128, E).transpose(1, 0, 2).reshape(128, EC * E))
    wqp, wkp, wvp = packw(Wq), packw(Wk), packw(Wv)
    idn = np.eye(BLK, dtype=np.float16)
    p = np.arange(BLK)[:, None]
    jj = np.arange(KSPAN)[None, :]
    band = (jj >= p) & (jj <= p + 2 * WHALF)
    band_add = np.where(band, np.float16(0.0), np.float16(MASK_NEG))
    mi = np.ascontiguousarray(
        np.concatenate([band_add, idn], axis=1))  # [128, 384]
    in_maps = []
    for c in range(NCORES):
        b, ci = divmod(c, NBLK)
        s = ci * CHUNK
        lo, hi = s - WHALF, s + CHUNK + WHALF
        a0, a1 = max(lo, 0), min(hi, L)
        xh = np.zeros((SPAN, E), np.float32)
        xh[a0 - lo:a1 - lo] = x[b, a0:a1]
        xT = np.ascontiguousarray(xh.T).astype(np.float16)  # [E, SPAN]
        xtp = np.ascontiguousarray(
            xT.reshape(EC, 128, SPAN).transpose(1, 0, 2).reshape(128, EC * SPAN))
        misc = np.zeros((128, 2 * EC + NBLK), np.float32)
        misc[:, 0:2 * EC:2] = bq.reshape(EC, 128).T
        misc[:, 1:2 * EC:2] = bk.reshape(EC, 128).T
        for i in range(NBLK):
            g = s - WHALF + i * BLK + jj  # global key index [1, KSPAN]
            n_invalid = (band & ((g < 0) | (g >= L))).sum(axis=1)
            misc[:, 2 * EC + i] = n_invalid.astype(np.float32)
        in_maps.append({
            "xtp": xtp, "wqp": wqp, "wkp": wkp, "wvp": wvp,
            "misc": misc, "mi": mi,
        })
    return in_maps


def kernel(x, Wq, bq, Wk, bk, Wv, bv, window_size, _trace=False):
    from concourse import bass_utils

    x = np.asarray(x, dtype=np.float32)
    Wq = np.asarray(Wq, dtype=np.float32)
    Wk = np.asarray(Wk, dtype=np.float32)
    Wv = np.asarray(Wv, dtype=np.float32)
    bq = np.asarray(bq, dtype=np.float32)
    bk = np.asarray(bk, dtype=np.float32)
    bv = np.asarray(bv, dtype=np.float32)
    assert int(window_size) == WHALF, f"kernel hardcodes window_size={WHALF}"
    assert x.shape == (B, L, E)
    # boundary-clip handling relies on padded keys scoring exp(0)=1 with zero
    # value vectors; that needs zero k/v biases (true for this problem).
    assert not np.any(bk) and not np.any(bv), "kernel requires bk == bv == 0"

    if "nc" not in _CACHE:
        _CACHE["nc"] = _build_bass()
    nc = _CACHE["nc"]

    in_maps = _host_inputs(x, Wq, bq, Wk, bk, Wv, bv)
    res = bass_utils.run_bass_kernel_spmd(
        nc, in_maps, core_ids=list(range(NCORES)), trace=_trace)
    _CACHE["last_results"] = res

    out = np.empty((B, L, E), np.float32)
    for c in range(NCORES):
        b, ci = divmod(c, NBLK)
        blk = res.results[c]["out"]  # [128, NBLK, E] block-major
        out[b, ci * CHUNK:(ci + 1) * CHUNK] = (
            blk.transpose(1, 0, 2).reshape(CHUNK, E).astype(np.float32))
    return out
